# revision 20
# baseline (speedup 1.0000x reference)
"""Channel-self-attention (LayerNorm + grouped-1x1-qkv + channel softmax attn
+ residual) on 8 TRN2 NeuronCores.

Strategy: shard the spatial axis (32^3 = 32768 -> 4096/core). Per core:
 - keep the x-shard [1024, 4096] f32 resident in SBUF
 - local stats (Sum x, Sum x^2) via DVE bn_stats/bn_aggr
 - u = gamma*x (ch 0..170 per batch) cast bf16, DMA-xbar-transpose SBUF->SBUF,
   PE Gram P^T[kap,a] = Sum_s u_{85+kap} u_a and t/g/h = u^T @ [gamma,beta,1]
 - ONE AllReduce (~132 KB) of (P^T, tgh, stats)
 - logits rebuilt from the Gram expansion of the LayerNorm algebra, softmax,
   apply att via one [89-row] matmul against [-gamma; beta; 1; gamma*x_V]
 - out = x + recip * PS  (softmax division folded into the epilogue)
"""
import sys

sys.path.insert(0, "/opt/trn_rl_repo")

import numpy as np
import ml_dtypes

B, C = 4, 256
S = 32 * 32 * 32          # 32768
NCORES = 8
SH = S // NCORES          # 4096 per-core spatial shard
NST = SH // 128           # 32 stiles
EPS = 1e-5
SCALE = float(S) ** -0.5

_BF = ml_dtypes.bfloat16

_cache = {}


def _build_program():
    """Trace the Bass/Tile program once; returns the compiled Bacc."""
    from contextlib import ExitStack
    import concourse.bass as bass
    import concourse.bacc as bacc
    import concourse.tile as tile
    from concourse import mybir, masks

    f32 = mybir.dt.float32
    bf16 = mybir.dt.bfloat16
    AF = mybir.ActivationFunctionType
    OP = mybir.AluOpType
    AX = mybir.AxisListType

    nc = bacc.Bacc(
        "TRN2",
        target_bir_lowering=False,
        debug=False,
        enable_asserts=False,
        num_devices=NCORES,
    )

    # ---------------- DRAM I/O ----------------
    xs_d = nc.dram_tensor("xs", [B * C, SH], f32, kind="ExternalInput")
    gl_d = nc.dram_tensor("gl", [1, SH], f32, kind="ExternalInput")
    gb1c_d = nc.dram_tensor("gb1c", [128, 3 * NST], bf16, kind="ExternalInput")
    gb1r_d = nc.dram_tensor("gb1r", [3, SH], bf16, kind="ExternalInput")
    eqt_d = nc.dram_tensor("eqt", [97, C], f32, kind="ExternalInput")
    ekt_d = nc.dram_tensor("ekt", [86, C], f32, kind="ExternalInput")
    w0_d = nc.dram_tensor("w0", [2 * 128, 87], bf16, kind="ExternalInput")
    bk_d = nc.dram_tensor("bk", [1, C], f32, kind="ExternalInput")
    sc_d = nc.dram_tensor("sc", [1, 8], f32, kind="ExternalInput")
    out_d = nc.dram_tensor("out", [B * C, SH], f32, kind="ExternalOutput")

    # Two bounce buffers so the first AllReduce (stats + batches 0,1)
    # overlaps stage-1 compute of batches 2,3.
    # Each PTK block is the [89, 89] merged matmul out:
    #   rows 0..2 = tghA^T (+3x3 junk corner), rows 3..88 = [P^T | tgh_K]
    PB = 89 * 89                   # 7921
    SX_OFF = 2 * PB                # in bncA
    SXX_OFF = SX_OFF + 1024
    TOT_A = SXX_OFF + 1024
    TOT_B = 2 * PB

    with tile.TileContext(nc) as tc, ExitStack() as ctx:
        const = ctx.enter_context(tc.tile_pool(name="const", bufs=1))
        xpool = ctx.enter_context(tc.tile_pool(name="xpool", bufs=1))
        upool = ctx.enter_context(tc.tile_pool(name="upool", bufs=1))
        utp = ctx.enter_context(tc.tile_pool(name="utp", bufs=4))
        rhsp = ctx.enter_context(tc.tile_pool(name="rhsp", bufs=2))
        osml = ctx.enter_context(tc.tile_pool(name="osml", bufs=2))
        small = ctx.enter_context(tc.tile_pool(name="small", bufs=2))
        dram = ctx.enter_context(tc.tile_pool(name="dram", bufs=1, space="DRAM"))

        # ------------- constants / inputs to SBUF -------------
        ident = const.tile([128, 128], f32)
        masks.make_identity(nc, ident[:])
        ident_bf = const.tile([128, 128], bf16)
        masks.make_identity(nc, ident_bf[:])
        gb1c_sb = const.tile([128, 3 * NST], bf16)
        nc.sync.dma_start(out=gb1c_sb[:], in_=gb1c_d.ap())
        eqt_sb = const.tile([97, C], f32)
        nc.sync.dma_start(out=eqt_sb[:], in_=eqt_d.ap())
        ekt_sb = const.tile([86, C], f32)
        nc.sync.dma_start(out=ekt_sb[:], in_=ekt_d.ap())
        w0_sb = const.tile([128, 2, 87], bf16)
        for jt in range(2):
            nc.sync.dma_start(out=w0_sb[:, jt, :], in_=w0_d[jt * 128:(jt + 1) * 128, :])
        def dram_bcast(dst, src_d, nparts, nfree):
            nc.gpsimd.dma_start(
                out=dst,
                in_=bass.AP(tensor=src_d, offset=0,
                            ap=[[0, nparts], [1, nfree]]))

        bk_bc = const.tile([128, C], f32)
        dram_bcast(bk_bc[:], bk_d, 128, C)
        sc_bc = const.tile([128, 8], f32)
        dram_bcast(sc_bc[:], sc_d, 128, 8)
        gam_bc = const.tile([128, SH], bf16)
        nc.gpsimd.dma_start(
            out=gam_bc[:],
            in_=bass.AP(tensor=gb1r_d, offset=SH,
                        ap=[[0, 128], [1, SH]]))
        nc.vector.tensor_scalar_mul(gam_bc[:], gam_bc[:], -1.0)

        # x resident: [128, 8, 4096] f32, tile t = rows t*128..t*128+127
        x_sb = xpool.tile([128, 8, SH], f32)
        for t in range(8):
            nc.sync.dma_start(out=x_sb[:, t, :], in_=xs_d[t * 128:(t + 1) * 128, :])

        # ------------- stage 1: stats via bn_stats -------------
        sums_sb = const.tile([128, 8], f32)
        sqs_sb = const.tile([128, 8], f32)
        for t in range(8):
            bnst = small.tile([128, 8, 6], f32, tag="bnst", bufs=1)
            for sub in range(8):
                nc.vector.bn_stats(
                    out=bnst[:, sub, :],
                    in_=x_sb[:, t, sub * 512:(sub + 1) * 512])
            aggr = small.tile([128, 2], f32, tag="aggr")
            nc.vector.bn_aggr(out=aggr[:], in_=bnst[:])
            nc.vector.tensor_scalar_mul(
                sums_sb[:, t:t + 1], aggr[:, 0:1], float(SH))
            tmp1 = small.tile([128, 1], f32, tag="tmp1")
            nc.vector.tensor_mul(tmp1[:], aggr[:, 0:1], aggr[:, 0:1])
            nc.vector.tensor_add(tmp1[:], tmp1[:], aggr[:, 1:2])
            nc.vector.tensor_scalar_mul(
                sqs_sb[:, t:t + 1], tmp1[:], float(SH))

        # ------------- stage 1: Gram + tgh per batch -------------
        bncA_in = dram.tile([TOT_A], f32)
        bncA_out = dram.tile([TOT_A], f32, addr_space="Shared")
        bncB_in = dram.tile([TOT_B], f32)
        bncB_out = dram.tile([TOT_B], f32, addr_space="Shared")

        u1s = []
        with tc.tile_pool(name="s1ps", bufs=2, space="PSUM") as stg1ps:
            for b in range(B):
                u0 = upool.tile([128, SH], bf16, tag="u0")
                # u1 covers the FULL second ctile (gamma*x, ch 128..255):
                # rows 0..42 feed the Gram transposes, rows 42..127 are
                # gamma*x_V reused for the M2 rhs (partition-shift DMA).
                u1 = upool.tile([128, SH], bf16, tag="u1", bufs=2)
                nc.vector.tensor_tensor(
                    out=u0[:], in0=x_sb[:, 2 * b, :], in1=gam_bc[:], op=OP.mult)
                nc.vector.tensor_tensor(
                    out=u1[:], in0=x_sb[:, 2 * b + 1, :],
                    in1=gam_bc[:], op=OP.mult)
                u1s.append(u1)

                ptk_ps = stg1ps.tile([89, 89], f32, tag="ptkps")
                for st in range(NST):
                    # PE transpose into PSUM bf16, copy back into the ut
                    # layout [A(86) | gamma beta 1 (3) | K(86)], then ONE
                    # matmul (lhsT = [gb1|K], rhs = [A|gb1]) produces
                    # tghA^T, P^T and tgh_K together in [89, 89].
                    tps = stg1ps.tile([128, 176], bf16, tag="tps", bufs=4)
                    ut = utp.tile([128, 175], bf16, name=f"ut_{b}_{st}", tag="ut")
                    sl = slice(st * 128, (st + 1) * 128)
                    nc.tensor.transpose(tps[:, 0:128], u0[:, sl], ident_bf[:])
                    nc.tensor.transpose(tps[:, 128:176], u1[0:48, sl],
                                        ident_bf[0:48, 0:48])
                    nc.scalar.copy(ut[:, 0:86], tps[:, 0:86])
                    nc.scalar.copy(ut[:, 89:175], tps[:, 85:171])
                    nc.gpsimd.tensor_copy(ut[:, 86:89],
                                          gb1c_sb[:, 3 * st:3 * st + 3])
                    nc.tensor.matmul(
                        ptk_ps[:], lhsT=ut[:, 86:175], rhs=ut[:, 0:89],
                        start=(st == 0), stop=(st == NST - 1))

                ptk_sb = small.tile([89, 89], f32, tag="ptksb", bufs=1)
                nc.scalar.copy(ptk_sb[:], ptk_ps[:])

                bnc = bncA_in if b < 2 else bncB_in
                po = (b % 2) * PB
                nc.gpsimd.dma_start(
                    out=bnc[po:po + PB].rearrange("(p f) -> p f", f=89),
                    in_=ptk_sb[:])
                if b == 1:
                    nc.gpsimd.dma_start(
                        out=bncA_in[SX_OFF:SX_OFF + 1024].rearrange(
                            "(t p) -> p t", p=128),
                        in_=sums_sb[:])
                    nc.gpsimd.dma_start(
                        out=bncA_in[SXX_OFF:SXX_OFF + 1024].rearrange(
                            "(t p) -> p t", p=128),
                        in_=sqs_sb[:])
                    nc.gpsimd.collective_compute(
                        "AllReduce", OP.add,
                        replica_groups=[list(range(NCORES))],
                        ins=[bncA_in[:].opt()], outs=[bncA_out[:].opt()])

            nc.gpsimd.collective_compute(
                "AllReduce", OP.add,
                replica_groups=[list(range(NCORES))],
                ins=[bncB_in[:].opt()], outs=[bncB_out[:].opt()])

        # ------------- DMA back -------------
        pt_back = const.tile([86, B, 86], f32)
        tga_back = const.tile([86, B, 3], f32)   # A-side: ch 0..85
        tgk_back = const.tile([86, B, 3], f32)   # K-side: ch 85..170
        for b in range(B):
            bout = bncA_out if b < 2 else bncB_out
            po = (b % 2) * PB
            nc.sync.dma_start(
                out=pt_back[:, b, :],
                in_=bass.AP(tensor=bout.tensor,
                            offset=bout.offset + po + 3 * 89,
                            ap=[[89, 86], [1, 86]]))
            nc.sync.dma_start(
                out=tgk_back[:, b, :],
                in_=bass.AP(tensor=bout.tensor,
                            offset=bout.offset + po + 3 * 89 + 86,
                            ap=[[89, 86], [1, 3]]))
            nc.sync.dma_start(
                out=tga_back[:, b, :],
                in_=bass.AP(tensor=bout.tensor,
                            offset=bout.offset + po,
                            ap=[[1, 86], [89, 3]]))
        # stats columns, all at partitions 0..85: [p, {Sx,Sxx}, {A,K,V}, b]
        sAK = const.tile([86, 2, 3, B], f32)
        for k, koff in ((0, SX_OFF), (1, SXX_OFF)):
            for g, goff in ((0, 0), (1, 85), (2, 170)):
                nc.sync.dma_start(
                    out=sAK[:, k, g, :],
                    in_=bass.AP(tensor=bncA_out.tensor,
                                offset=bncA_out.offset + koff + goff,
                                ap=[[1, 86], [256, B]]))

        # ------------- stage 2/3 (phase-interleaved in pairs) -------------
        psA = ctx.enter_context(tc.tile_pool(name="psA", bufs=2, space="PSUM"))
        psB = ctx.enter_context(tc.tile_pool(name="psB", bufs=3, space="PSUM"))
        psC = ctx.enter_context(tc.tile_pool(name="psC", bufs=3, space="PSUM"))

        invS = 1.0 / float(S)
        st2 = [dict() for _ in range(B)]

        def phase_rhs(b):
            # rhs_M2 [128, SH] bf16: rows 0..85 = gamma*x_V from u1 via a
            # DRAM round-trip (a direct SBUF->SBUF DMA would deadlock
            # against concurrent xbar transposes), rows 86..88 =
            # [ones, -gamma, beta]
            rhs_m2 = rhsp.tile([128, SH], bf16, tag="rhsm2", name=f"rhs{b}")
            nc.gpsimd.dma_start(out=rhs_m2[0:86, :], in_=u1s[b][42:128, :])
            nc.gpsimd.dma_start(out=rhs_m2[86:89, :], in_=gb1r_d.ap())
            st2[b]["rhs"] = rhs_m2

        def phase_vec(b):
            s = st2[b]
            mAK = small.tile([86, 3], f32, tag="mAK", name=f"mAK{b}")
            nc.vector.tensor_scalar(
                out=mAK[:], in0=sAK[:, 0, :, b], scalar1=invS, scalar2=None,
                op0=OP.mult)
            vAK = small.tile([86, 3], f32, tag="vAK", name=f"vAK{b}")
            nc.vector.tensor_scalar(
                out=vAK[:], in0=sAK[:, 1, :, b], scalar1=invS, scalar2=EPS,
                op0=OP.mult, op1=OP.add)
            msq = small.tile([86, 3], f32, tag="msq", name=f"msq{b}")
            nc.vector.tensor_mul(msq[:], mAK[:], mAK[:])
            nc.vector.tensor_sub(vAK[:], vAK[:], msq[:])
            nc.scalar.activation(out=vAK[:], in_=vAK[:], func=AF.Sqrt)
            rAK = small.tile([86, 3], f32, tag="rAK", name=f"rAK{b}")
            nc.vector.reciprocal(rAK[:], vAK[:])
            invrV = small.tile([86, 1], f32, tag="invrV", name=f"invrV{b}")
            nc.vector.reciprocal(invrV[:], rAK[:, 2:3])
            mvinv_bf = small.tile([86, 2], bf16, tag="mvinv", name=f"mvinv{b}")
            nc.vector.tensor_copy(mvinv_bf[:, 0:1], mAK[:, 2:3])
            nc.vector.tensor_copy(mvinv_bf[:, 1:2], invrV[:])
            rv_ext = small.tile([128, 1], f32, tag="rvext", name=f"rvext{b}")
            nc.vector.memset(rv_ext[64:128, :], 1.0)
            nc.vector.tensor_copy(rv_ext[0:86, :], rAK[:, 2:3])
            s["mAK"], s["rAK"] = mAK, rAK
            s["mvinv"], s["rvext"] = mvinv_bf, rv_ext

            tA = tga_back[:, b, 0:1]
            gA = tga_back[:, b, 1:2]
            hA = tga_back[:, b, 2:3]
            tK = tgk_back[:, b, 0:1]
            gK = tgk_back[:, b, 1:2]
            hK = tgk_back[:, b, 2:3]
            mA, mK = mAK[:, 0:1], mAK[:, 1:2]
            rA, rK = rAK[:, 0:1], rAK[:, 1:2]
            scG1 = sc_bc[0:86, 0:1]
            scG2 = sc_bc[0:86, 1:2]
            scGb = sc_bc[0:86, 2:3]
            scB1 = sc_bc[0:86, 3:4]

            ntK = small.tile([86, 1], f32, tag="ntK", name=f"ntK{b}")
            nc.vector.tensor_scalar_mul(ntK[:], tK, -1.0)
            nmK = small.tile([86, 1], f32, tag="nmK", name=f"nmK{b}")
            nc.vector.tensor_scalar_mul(nmK[:], mK, -1.0)
            g2mK = small.tile([86, 1], f32, tag="g2mK", name=f"g2mK{b}")
            nc.vector.tensor_scalar(
                out=g2mK[:], in0=mK, scalar1=scG2, scalar2=None, op0=OP.mult)
            t3c = small.tile([86, 1], f32, tag="t3c", name=f"t3c{b}")
            nc.vector.tensor_scalar(
                out=t3c[:], in0=mK, scalar1=scGb, scalar2=None, op0=OP.mult)
            nc.vector.tensor_sub(t3c[:], gK, t3c[:])
            nc.vector.tensor_mul(t3c[:], rK, t3c[:])
            t2c = small.tile([86, 1], f32, tag="t2c", name=f"t2c{b}")
            nc.vector.tensor_scalar(
                out=t2c[:], in0=mA, scalar1=scGb, scalar2=None, op0=OP.mult)
            nc.vector.tensor_sub(t2c[:], gA, t2c[:])
            nc.vector.tensor_mul(t2c[:], rA, t2c[:])
            syA = small.tile([86, 1], f32, tag="syA", name=f"syA{b}")
            nc.vector.tensor_scalar(
                out=syA[:], in0=mA, scalar1=scG1, scalar2=None, op0=OP.mult)
            nc.vector.tensor_sub(syA[:], hA, syA[:])
            nc.vector.tensor_mul(syA[:], rA, syA[:])
            nc.vector.tensor_scalar(
                out=syA[:], in0=syA[:], scalar1=scB1, scalar2=None, op0=OP.add)
            syK = small.tile([86, 1], f32, tag="syK", name=f"syK{b}")
            nc.vector.tensor_scalar(
                out=syK[:], in0=mK, scalar1=scG1, scalar2=None, op0=OP.mult)
            nc.vector.tensor_sub(syK[:], hK, syK[:])
            nc.vector.tensor_mul(syK[:], rK, syK[:])
            nc.vector.tensor_scalar(
                out=syK[:], in0=syK[:], scalar1=scB1, scalar2=None, op0=OP.add)
            s["ntK"], s["nmK"], s["g2mK"] = ntK, nmK, g2mK
            s["t3c"], s["syA"], s["syK"] = t3c, syA, syK

            # rows (mA, tA, rA, term2) -> transpose -> DRAM -> one bcast DMA
            pack = small.tile([86, 4], f32, tag="pack", name=f"pack{b}")
            nc.vector.tensor_copy(pack[:, 0:1], mA)
            nc.vector.tensor_copy(pack[:, 1:2], tA)
            nc.vector.tensor_copy(pack[:, 2:3], rA)
            nc.vector.tensor_copy(pack[:, 3:4], t2c[:])
            packT_ps = psA.tile([4, 86], f32, tag="psA", name=f"pT{b}")
            nc.tensor.transpose(packT_ps[:], pack[:], ident[0:86, 0:86])
            packT = small.tile([4, 86], f32, tag="packT", name=f"packT{b}")
            nc.scalar.copy(packT[:], packT_ps[:])
            rows_d = dram.tile([4, 86], f32, name=f"rowsd{b}", tag="rowsd",
                               bufs=2)
            nc.gpsimd.dma_start(out=rows_d[:], in_=packT[:])
            bc4 = small.tile([86, 4, 86], f32, tag="bc4", name=f"bc4{b}")
            nc.gpsimd.dma_start(
                out=bc4[:],
                in_=bass.AP(tensor=rows_d.tensor, offset=rows_d.offset,
                            ap=[[0, 86], [86, 4], [1, 86]]))
            s["bc4"] = bc4

        def phase_syy(b):
            s = st2[b]
            bc4 = s["bc4"]
            rK = s["rAK"][:, 1:2]
            scBb = sc_bc[0:86, 4:5]
            syy = small.tile([86, 97], f32, tag="syy", name=f"syy{b}")
            nc.vector.memset(syy[:, 86:96], 0.0)
            nc.vector.scalar_tensor_tensor(
                out=syy[:, 0:86], in0=bc4[:, 0, :], scalar=s["ntK"][:],
                in1=pt_back[:, b, :], op0=OP.mult, op1=OP.add)
            nc.vector.scalar_tensor_tensor(
                out=syy[:, 0:86], in0=bc4[:, 1, :], scalar=s["nmK"][:],
                in1=syy[:, 0:86], op0=OP.mult, op1=OP.add)
            nc.vector.scalar_tensor_tensor(
                out=syy[:, 0:86], in0=bc4[:, 0, :], scalar=s["g2mK"][:],
                in1=syy[:, 0:86], op0=OP.mult, op1=OP.add)
            nc.vector.scalar_tensor_tensor(
                out=syy[:, 0:86], in0=bc4[:, 2, :], scalar=rK,
                in1=syy[:, 0:86], op0=OP.mult, op1=OP.mult)
            nc.vector.tensor_add(syy[:, 0:86], syy[:, 0:86], bc4[:, 3, :])
            nc.vector.tensor_scalar(
                out=syy[:, 0:86], in0=syy[:, 0:86], scalar1=s["t3c"][:],
                scalar2=scBb, op0=OP.add, op1=OP.add)
            nc.vector.tensor_copy(syy[:, 96:97], s["syK"][:])
            s["syy"] = syy

        def phase_logits(b):
            s = st2[b]
            u_ps = psA.tile([97, C], f32, tag="psA", name=f"ups{b}")
            nc.tensor.matmul(u_ps[:], lhsT=s["syy"][:], rhs=ekt_sb[:],
                             start=True, stop=True)
            u_ext = small.tile([128, C], f32, tag="uext", name=f"uext{b}")
            nc.vector.memset(u_ext[64:128, :], 0.0)
            nc.vector.scalar_tensor_tensor(
                out=u_ext[0:86, :], in0=bk_bc[0:86, :], scalar=s["syA"][:],
                in1=u_ps[0:86, :], op0=OP.mult, op1=OP.add)
            nc.vector.tensor_scalar_mul(
                u_ext[96:97, :], bk_bc[96:97, :], float(S))
            nc.vector.tensor_add(u_ext[96:97, :], u_ext[96:97, :],
                                 u_ps[96:97, :])

            att_sb = []
            recip2 = small.tile([128, 2], f32, tag="recip2", name=f"re{b}")
            z2 = small.tile([128, 2], f32, tag="z2", name=f"z2{b}")
            for it in range(2):
                log_ps = psB.tile([128, 512], f32, tag="psB", name=f"lg{b}{it}")
                nc.tensor.matmul(
                    log_ps[:, 0:C], lhsT=eqt_sb[:, it * 128:(it + 1) * 128],
                    rhs=u_ext[0:97, :], start=True, stop=True)
                rmax = small.tile([128, 1], f32, tag="rmax", name=f"rm{b}{it}")
                nc.vector.reduce_max(rmax[:], log_ps[:, 0:C], axis=AX.X)
                nbias = small.tile([128, 1], f32, tag="nbias",
                                   name=f"nb{b}{it}")
                nc.vector.tensor_scalar_mul(nbias[:], rmax[:], -SCALE)
                a_sb = small.tile([128, C], f32, tag=f"attsb{it}",
                                  name=f"att{b}{it}")
                nc.scalar.activation(
                    out=a_sb[:], in_=log_ps[:, 0:C], func=AF.Exp,
                    bias=nbias[:], scale=SCALE, accum_out=z2[:, it:it + 1])
                nc.vector.reciprocal(recip2[:, it:it + 1], z2[:, it:it + 1])
                att_sb.append(a_sb)
            s["att"], s["recip2"] = att_sb, recip2

        def phase_nt(b):
            s = st2[b]
            ntc_ps = psC.tile([128, C], f32, tag="psC", name=f"ntc{b}")
            for jt in range(2):
                at_ps = psC.tile([128, C], f32, tag="psC", name=f"atp{b}{jt}")
                for it in range(2):
                    nc.tensor.transpose(
                        at_ps[:, it * 128:(it + 1) * 128],
                        s["att"][it][:, jt * 128:(jt + 1) * 128],
                        ident[:])
                at_bf = small.tile([128, C], bf16, tag=f"atbf{jt}",
                                   name=f"atb{b}{jt}")
                nc.scalar.copy(at_bf[:], at_ps[:])
                nc.tensor.matmul(
                    ntc_ps[0:87, :], lhsT=w0_sb[:, jt, :], rhs=at_bf[:],
                    start=(jt == 0), stop=(jt == 1))

            # lhsT_M2 [128, C] bf16: rows 0..85=NR, 86=cv, 87=c1, 88=c2.
            # rv_ext has 1.0 at row 86 so cv copies through unscaled.
            lhs_m2 = small.tile([128, C], bf16, tag="lhsm2", name=f"lm{b}")
            rv = s["rvext"]
            nc.scalar.activation(
                out=lhs_m2[0:64, :], in_=ntc_ps[0:64, :], func=AF.Copy,
                scale=rv[0:64, :])
            nc.scalar.activation(
                out=lhs_m2[64:87, :], in_=ntc_ps[64:87, :], func=AF.Copy,
                scale=rv[64:87, :])
            nc.tensor.matmul(
                ntc_ps[64:66, :], lhsT=s["mvinv"][:],
                rhs=lhs_m2[0:86, :], start=True, stop=True)
            c12_sb = small.tile([128, C], bf16, tag="c12sb", name=f"c12{b}")
            nc.scalar.copy(c12_sb[64:66, :], ntc_ps[64:66, :])
            nc.gpsimd.dma_start(out=lhs_m2[87:89, :], in_=c12_sb[64:66, :])
            s["lhs_m2"] = lhs_m2

        def phase_m2(b):
            s = st2[b]
            lhs_m2, rhs_m2, recip2 = s["lhs_m2"], s["rhs"], s["recip2"]
            for it in range(2):
                for ch in range(8):
                    ostg = osml.tile([128, 512], f32, tag="ostg", bufs=3,
                                     name=f"o{b}{it}{ch}")
                    o_ps = psB.tile([128, 512], f32, tag="psB",
                                    name=f"op{b}{it}{ch}")
                    nc.tensor.matmul(
                        o_ps[:],
                        lhsT=lhs_m2[0:89, it * 128:(it + 1) * 128],
                        rhs=rhs_m2[0:89, ch * 512:(ch + 1) * 512],
                        start=True, stop=True)
                    nc.vector.scalar_tensor_tensor(
                        out=ostg[:], in0=o_ps[:],
                        scalar=recip2[:, it:it + 1],
                        in1=x_sb[:, 2 * b + it, ch * 512:(ch + 1) * 512],
                        op0=OP.mult, op1=OP.add)
                    nc.sync.dma_start(
                        out=out_d[(2 * b + it) * 128:(2 * b + it + 1) * 128,
                                  ch * 512:(ch + 1) * 512],
                        in_=ostg[:])

        phases = [phase_rhs, phase_vec, phase_syy, phase_logits, phase_nt,
                  phase_m2]
        for pair in ((0, 1), (2, 3)):
            for ph in phases:
                for b in pair:
                    ph(b)

    nc.compile()
    return nc


def _host_prep(x, gamma, beta, w_qkv, b_qkv):
    xf = np.ascontiguousarray(np.asarray(x, np.float32).reshape(B * C, S))
    gam = np.asarray(gamma, np.float32).reshape(-1)
    bet = np.asarray(beta, np.float32).reshape(-1)
    w_qkv = np.asarray(w_qkv, np.float32)
    b_qkv = np.asarray(b_qkv, np.float32)
    w_q, w_k, w_v = w_qkv[:C], w_qkv[C:2 * C], w_qkv[2 * C:]
    b_q, b_k, b_v = b_qkv[:C], b_qkv[C:2 * C], b_qkv[2 * C:]

    ii = np.arange(C)
    eqt = np.zeros((97, C), np.float32)
    eqt[ii // 3, ii] = w_q
    eqt[96] = b_q
    ekt = np.zeros((86, C), np.float32)
    ekt[(C + ii) // 3 - 85, ii] = w_k
    w0 = np.zeros((C, 87), np.float32)
    w0[ii, (2 * C + ii) // 3 - 170] = w_v
    w0[:, 86] = b_v
    w0 = w0.astype(_BF)

    sc = np.zeros((1, 8), np.float32)
    sc[0, :5] = [gam.sum(), (gam * gam).sum(), (gam * bet).sum(),
                 bet.sum(), (bet * bet).sum()]

    in_maps = []
    for r in range(NCORES):
        sl = slice(r * SH, (r + 1) * SH)
        gl = gam[sl]
        bl = bet[sl]
        gb1c = np.empty((128, 3 * NST), np.float32)
        for st in range(NST):
            gb1c[:, 3 * st] = gl[st * 128:(st + 1) * 128]
            gb1c[:, 3 * st + 1] = bl[st * 128:(st + 1) * 128]
            gb1c[:, 3 * st + 2] = 1.0
        gb1r = np.stack([np.ones(SH, np.float32), -gl, bl], 0)
        in_maps.append({
            "xs": np.ascontiguousarray(xf[:, sl]),
            "gl": gl.reshape(1, SH).copy(),
            "gb1c": gb1c.astype(_BF),
            "gb1r": gb1r.astype(_BF),
            "eqt": eqt,
            "ekt": ekt,
            "w0": w0,
            "bk": b_k.reshape(1, C).copy(),
            "sc": sc,
        })
    return in_maps


def kernel(x, gamma, beta, w_qkv, b_qkv):
    from concourse.bass_utils import run_bass_kernel_spmd

    if "nc" not in _cache:
        _cache["nc"] = _build_program()
    nc = _cache["nc"]

    in_maps = _host_prep(x, gamma, beta, w_qkv, b_qkv)
    res = run_bass_kernel_spmd(nc, in_maps, core_ids=list(range(NCORES)))
    out = np.empty((B * C, S), np.float32)
    for r in range(NCORES):
        out[:, r * SH:(r + 1) * SH] = res.results[r]["out"]
    return out.reshape(np.asarray(x).shape)


if __name__ == "__main__":
    rng = np.random.default_rng(0)
    inputs = {
        "x": rng.standard_normal((B, C, 32, 32, 32)).astype(np.float32),
        "gamma": (1 + 0.1 * rng.standard_normal((32, 32, 32))).astype(np.float32),
        "beta": (0.1 * rng.standard_normal((32, 32, 32))).astype(np.float32),
        "w_qkv": (0.5 * rng.standard_normal(3 * C)).astype(np.float32),
        "b_qkv": (0.05 * rng.standard_normal(3 * C)).astype(np.float32),
    }
    o = kernel(**inputs)
    print("out", o.shape, o.dtype, float(np.abs(o).mean()))
